# revision 5
# baseline (speedup 1.0000x reference)
"""Trainium2 Bass kernel for CompositionalPhoneticsModel (segment_reduce).

Computation (reference):
    phone   = einsum('bth,hp->btp', enc_output, feature2phone) / sqrt(H)
    allo    = where(mapping>0, phone[:,:,None,:]*mapping, -inf)   # mapping is 0/1
    phoneme = max(allo, axis=-1)                                  # masked segment max
    out     = log_softmax(phoneme, axis=2)

Device strategy (8 NeuronCores, data-parallel over the B*T=8192 rows):
  * Host gathers feature2phone columns into segment-contiguous order
    (phones in 2 segments get duplicated columns; NNZ ~ 506 after even-pad),
    folds in the 1/sqrt(H) scale, and sorts segments by length so the
    per-segment max is a handful of strided DVE reduce_max ops.  The device
    phoneme order is a permutation of 0..95; max/logsumexp are
    permutation-invariant so the host un-permutes the output at the end.
  * enc is pre-cast to bf16 and pre-interleaved per 256-row megatile as
    [128, NH, TW] (chunk-major) so a megatile DMA is one 2560B line per
    partition AND a single contraction chunk is a contiguous 512B line —
    the head loads are chunk-split so the first matmul starts as soon as
    (wk chunk 0, enc chunk 0) land instead of after the full tiles.
  * The PE p-state ramps with CONTINUOUS use (~3.7us to full speed; an idle
    gap resets it).  A stream of small warmup matmuls keeps the PE busy
    from kernel start until the real weights arrive.
  * log-softmax without max-subtraction (phone logits are ~N(0,1); exp fits
    fp32 comfortably): Exp on ScalarE (one joint Exp/Ln table load), row
    sums via DVE reduce_sum, Ln on ScalarE, final x - ln(sum) on GpSimd
    (otherwise idle) so the DVE only does the segment maxes + sums.
  * The last megatile's post-chain is split per 128-row block so its
    reduce/softmax overlaps the megatile's own second-half matmuls.
"""

from contextlib import ExitStack

import numpy as np
import ml_dtypes

import concourse.bass as bass
import concourse.bacc as bacc
import concourse.tile as tile
from concourse import mybir
from concourse.bass_utils import run_bass_kernel_spmd

B, T, H = 8, 1024, 640
N_PHONEME, N_PHONE = 96, 230
N_CORES = 8
ROWS = B * T
RC = ROWS // N_CORES          # rows per core
NH = H // 128                 # contraction chunks
TW = 256                      # rows per enc megatile
RT = TW // 128                # row blocks per megatile
NMT = RC // TW                # megatiles per core
NB = RC // 128                # 128-row blocks per core
NWARM = 16                    # PE-ramp warmup matmuls (128 cols each)
BF16 = ml_dtypes.bfloat16


def _structure(mapping: np.ndarray):
    """Segment-contiguous gather order, grouped by segment length (desc).

    Returns (col_ids, groups, perm):
      col_ids: phone index feeding each device matmul column (len NNZ)
      groups:  list of (L, nL, col_off, out_off) — nL segments of length L
               occupy matmul cols [col_off, col_off+nL*L) and device output
               cols [out_off, out_off+nL)
      perm:    perm[j] = original phoneme id of device output column j
    """
    segs = [np.nonzero(mapping[m] > 0)[0] for m in range(N_PHONEME)]
    assert min(len(s) for s in segs) >= 1
    # pad segment lengths up to even targets (repeating a member doesn't
    # change the max): fewer distinct lengths -> fewer DVE reduce ops.
    # Only worthwhile while the matmul width stays within one PSUM bank.
    padded = []
    for s in segs:
        t = ((len(s) + 1) // 2) * 2
        padded.append(np.concatenate([s, np.full(t - len(s), s[0], s.dtype)]))
    if sum(len(s) for s in padded) <= 512:
        segs = padded
    lengths = np.array([len(s) for s in segs])
    order = np.argsort(-lengths, kind="stable")
    col_ids, groups, perm = [], [], []
    i = 0
    while i < N_PHONEME:
        L = int(lengths[order[i]])
        j = i
        while j < N_PHONEME and lengths[order[j]] == L:
            j += 1
        groups.append((L, j - i, len(col_ids), i))
        for k in range(i, j):
            m = int(order[k])
            col_ids.extend(segs[m].tolist())
            perm.append(m)
        i = j
    return np.array(col_ids, dtype=np.int64), groups, np.array(perm, dtype=np.int64)


def _patch_act_tables():
    """Make Exp and Ln resolve to the same activation-table set.

    bacc's insert_act_table_loads models a single table slot, so a kernel
    alternating Exp/Ln reloads a 1.3us table on every transition.  act_info
    has a joint set ('natural_log_exp_and_others') containing both; keep the
    set list's order/indices intact but strip Exp/Ln from the other sets so
    the pass picks the joint set for both and emits a single load.
    """
    if getattr(bacc, "_act_tables_patched", False):
        return
    from concourse import hw_specs
    orig = hw_specs.get_activation_tables
    act = mybir.ActivationFunctionType

    def patched(module_arch):
        tabs = orig(module_arch)
        joint = [k for k, v in tabs.items() if act.Exp in v and act.Ln in v]
        if not joint:
            return tabs
        j = joint[0]
        return {
            k: (v if k == j else (v - {act.Exp, act.Ln}))
            for k, v in tabs.items()
        }

    bacc.get_activation_tables = patched
    bacc._act_tables_patched = True


def _build_program(nnz: int, groups):
    """Build + compile the per-core Bass program. Returns the Bacc object."""
    _patch_act_tables()
    nc = bacc.Bacc("TRN2", target_bir_lowering=False, debug=False)
    dt = mybir.dt
    act = mybir.ActivationFunctionType
    X = mybir.AxisListType.X

    # enc chunk-major per megatile: element (mt, p, c, t) = enc[mt*TW+t, c*128+p]
    enck_d = nc.dram_tensor("enck", [NMT, 128, NH, TW], dt.bfloat16, kind="ExternalInput")
    # W interleaved: [128, NH, nnz]; element (p, c, n) = W[c*128+p, n]
    wk_d = nc.dram_tensor("wk", [128, NH, nnz], dt.bfloat16, kind="ExternalInput")
    # out packed: [128, NB, 96]; element (p, b, m) = out[b*128+p, m]
    out_d = nc.dram_tensor("out", [128, NB, N_PHONEME], dt.float32, kind="ExternalOutput")

    with ExitStack() as ctx:
        tc = ctx.enter_context(tile.TileContext(nc))
        wpool = ctx.enter_context(tc.tile_pool(name="wpool", bufs=1))
        epool = ctx.enter_context(tc.tile_pool(name="epool", bufs=4))
        ppool = ctx.enter_context(tc.tile_pool(name="ppool", bufs=4, space="PSUM"))
        spool = ctx.enter_context(tc.tile_pool(name="spool", bufs=2))

        wt = wpool.tile([128, NH, nnz], dt.bfloat16)
        ets = [epool.tile([128, NH, TW], dt.bfloat16, tag="et", name=f"et{i}")
               for i in range(NMT)]

        # Head DMAs, chunk-split and spread over both HWDGE queues so the
        # first matmul's inputs (wk c0, enc mt0 c0) land first:
        #   Sync  q: wk c0 | wk c1-4 | et1 c0-1 | et1 c2-4 | out0..3
        #   Scalar q: et0 c0 | et0 c1-4 | et2 | et3
        nc.sync.dma_start(wt[:, 0, :], wk_d[:, 0, :])
        nc.scalar.dma_start(ets[0][:, 0, :], enck_d[0, :, 0, :])
        nc.sync.dma_start(wt[:, 1:, :], wk_d[:, 1:, :])
        nc.scalar.dma_start(ets[0][:, 1:, :], enck_d[0, :, 1:, :])
        nc.sync.dma_start(ets[1][:, 0:2, :], enck_d[1, :, 0:2, :])
        nc.sync.dma_start(ets[1][:, 2:, :], enck_d[1, :, 2:, :])
        nc.scalar.dma_start(ets[2][:], enck_d[2])
        nc.scalar.dma_start(ets[3][:], enck_d[3])

        # PE warmup: small dummy matmuls keep the tensor engine continuously
        # busy (ramping its p-state) until the real weights land.  They write
        # the first megatile's PSUM bank; the real c==0 matmul (start=True)
        # overwrites it.
        wu = wpool.tile([128, 128], dt.bfloat16)
        nc.gpsimd.memset(wu[:], 0.0)
        pss = [ppool.tile([128, RT, 512], dt.float32, tag="ps", name=f"ps{i}")
               for i in range(NMT)]
        for _ in range(NWARM):
            nc.tensor.matmul(pss[0][:, 0, :128], wu[:], wu[:], start=True, stop=True)

        def mm(mt, r, c):
            nc.tensor.matmul(
                pss[mt][:, r, :nnz],
                ets[mt][:, c, r * 128:(r + 1) * 128],
                wt[:, c, :],
                start=(c == 0),
                stop=(c == NH - 1),
            )

        def seg_max(pmax, ps, mt, r=None):
            """Segment max via one strided DVE reduce per length group."""
            for (L, nL, coff, ooff) in groups:
                if r is None:
                    src = ps[:, :, coff:coff + nL * L].rearrange(
                        "p r (s l) -> p r s l", l=L)
                    dst = pmax[:, :, ooff:ooff + nL]
                else:
                    src = ps[:, r, coff:coff + nL * L].rearrange(
                        "p (s l) -> p s l", l=L)
                    dst = pmax[:, r, ooff:ooff + nL]
                nc.vector.reduce_max(dst, src, axis=X)

        def softmax_tail(pmax, ex, se, lse, ott, rs):
            """exp (ScalarE) -> row sums (DVE) -> ln (ScalarE) -> sub (GpSimd)."""
            if isinstance(rs, int):
                rows = [rs]
                rs = slice(rs, rs + 1)
            else:
                rows = list(range(RT))
            nc.scalar.activation(ex[:, rs, :], pmax[:, rs, :], act.Exp)
            nc.vector.reduce_sum(se[:, rs], ex[:, rs, :], axis=X)
            nc.scalar.activation(lse[:, rs], se[:, rs], act.Ln)
            for r in rows:
                nc.gpsimd.tensor_scalar_sub(
                    ott[:, r, :], pmax[:, r, :], lse[:, r:r + 1])

        def post_tiles(mt):
            pmax = spool.tile([128, RT, N_PHONEME], dt.float32, tag="pmax")
            ex = spool.tile([128, RT, N_PHONEME], dt.float32, tag="ex")
            se = spool.tile([128, RT], dt.float32, tag="se")
            lse = spool.tile([128, RT], dt.float32, tag="lse")
            ott = spool.tile([128, RT, N_PHONEME], dt.float32, tag="ott")
            return pmax, ex, se, lse, ott

        # megatile 0: c-outer so each chunk's matmuls start as soon as that
        # chunk's weight+enc DMAs land (interleaves the r0/r1 PSUM
        # accumulation groups, which is fine: accumulate flags are
        # per-address).
        for c in range(NH):
            for r in range(RT):
                mm(0, r, c)
        pmax, ex, se, lse, ott = post_tiles(0)
        seg_max(pmax, pss[0], 0)
        softmax_tail(pmax, ex, se, lse, ott, slice(None))
        nc.sync.dma_start(out_d[:, 0:RT, :], ott[:])

        # middle megatiles: r-outer, batched post-chain
        for mt in range(1, NMT - 1):
            for r in range(RT):
                for c in range(NH):
                    mm(mt, r, c)
            pmax, ex, se, lse, ott = post_tiles(mt)
            seg_max(pmax, pss[mt], mt)
            softmax_tail(pmax, ex, se, lse, ott, slice(None))
            nc.sync.dma_start(out_d[:, mt * RT:(mt + 1) * RT, :], ott[:])

        # last megatile: r-outer with the post-chain split per row block so
        # r0's reduce/softmax overlaps r1's matmuls (shorter exposed tail).
        mt = NMT - 1
        pmax, ex, se, lse, ott = post_tiles(mt)
        for r in range(RT):
            for c in range(NH):
                mm(mt, r, c)
            seg_max(pmax, pss[mt], mt, r=r)
            softmax_tail(pmax, ex, se, lse, ott, r)
        nc.sync.dma_start(out_d[:, mt * RT:(mt + 1) * RT, :], ott[:])

    nc.compile()
    return nc


_CACHE: dict = {}


def _get_compiled(mapping: np.ndarray):
    key = mapping.astype(np.float32).tobytes()
    if _CACHE.get("key") != key:
        col_ids, groups, perm = _structure(mapping)
        nc = _build_program(len(col_ids), groups)
        _CACHE.update(key=key, col_ids=col_ids, groups=groups, perm=perm, nc=nc)
    return _CACHE["nc"], _CACHE["col_ids"], _CACHE["perm"]


def _prep_in_maps(enc_output, feature2phone, col_ids):
    scale = np.float32(1.0) / np.sqrt(np.float32(H))
    wg = (feature2phone.astype(np.float32) * scale)[:, col_ids].astype(BF16)
    # [H, nnz] -> [128, NH, nnz]
    wk = np.ascontiguousarray(wg.reshape(NH, 128, -1).transpose(1, 0, 2))
    # enc [ROWS, H] -> per-core [NMT, 128, NH, TW] (chunk-major megatiles)
    e4 = enc_output.astype(BF16).reshape(N_CORES, NMT, TW, NH, 128)
    enck = np.ascontiguousarray(e4.transpose(0, 1, 4, 3, 2))
    in_maps = []
    for c in range(N_CORES):
        in_maps.append({"enck": enck[c], "wk": wk})
    return in_maps


def run_device(enc_output, feature2phone, mapping, trace=False, **kw):
    """Build/compile (cached), run on the 8 cores, return (output, BassKernelResults)."""
    enc_output = np.asarray(enc_output)
    feature2phone = np.asarray(feature2phone)
    mapping = np.asarray(mapping)
    nc, col_ids, perm = _get_compiled(mapping)
    in_maps = _prep_in_maps(enc_output, feature2phone, col_ids)
    res = run_bass_kernel_spmd(
        nc, in_maps, core_ids=list(range(N_CORES)), trace=trace, **kw
    )
    # device out [128, NB, 96] packed -> rows b*128+p
    dev = np.concatenate(
        [res.results[c]["out"].transpose(1, 0, 2).reshape(RC, N_PHONEME)
         for c in range(N_CORES)],
        axis=0,
    )
    out = np.empty_like(dev)
    out[:, perm] = dev
    return out.reshape(B, T, N_PHONEME).astype(np.float32), res


def kernel(enc_output, feature2phone, mapping):
    out, _ = run_device(enc_output, feature2phone, mapping)
    return out


# revision 6
# speedup vs baseline: 1.2252x; 1.2252x over previous
"""Trainium2 Bass kernel for CompositionalPhoneticsModel (segment_reduce).

Computation (reference):
    phone   = einsum('bth,hp->btp', enc_output, feature2phone) / sqrt(H)
    allo    = where(mapping>0, phone[:,:,None,:]*mapping, -inf)   # mapping is 0/1
    phoneme = max(allo, axis=-1)                                  # masked segment max
    out     = log_softmax(phoneme, axis=2)

Device strategy (8 NeuronCores, data-parallel over the B*T=8192 rows):
  * Host gathers feature2phone columns into segment-contiguous order
    (phones in 2 segments get duplicated columns; NNZ ~ 506 after even-pad),
    folds in the 1/sqrt(H) scale, and sorts segments by length so the
    per-segment max is a handful of strided DVE reduce_max ops.  The device
    phoneme order is a permutation of 0..95; max/logsumexp are
    permutation-invariant so the host un-permutes the output at the end.
  * DMA on TRN2 is descriptor-rate limited: each HWDGE queue retires ~60-90
    descriptors/us and a full 128-partition transfer is 128 descriptors
    (~1.5-2us) REGARDLESS of bytes.  So every big transfer is split into
    partition halves issued on both HWDGE queues (Sync + Scalar) in
    parallel, and tiles are kept as large as SBUF allows so lines are long.
  * enc is pre-cast to bf16 and pre-interleaved per row-tile as
    [128, NH, rows] (chunk-major) so a tile's per-partition line is one
    contiguous run (64 descriptors per half-DMA).
  * The PE p-state ramps with CONTINUOUS use (~3.7us to full speed; an idle
    gap resets it).  A stream of small warmup matmuls keeps the PE busy
    from kernel start until the real weights arrive.
  * log-softmax without max-subtraction (phone logits are ~N(0,1); exp fits
    fp32 comfortably): Exp on ScalarE, row sums via DVE reduce_sum, 1/sum
    via DVE reciprocal, Ln(1/sum) = -lse on ScalarE, and the final
    x + (-lse) as a ScalarE Identity activation with per-partition bias
    (Identity/Exp/Ln live in one activation table -> single table load).
  * The last 2 row blocks are separate 128-row tiles so the exposed tail
    (reduce -> softmax -> out DMA of the final block) is half as deep.
"""

from contextlib import ExitStack

import numpy as np
import ml_dtypes

import concourse.bass as bass
import concourse.bacc as bacc
import concourse.tile as tile
from concourse import mybir
from concourse.bass_utils import run_bass_kernel_spmd

B, T, H = 8, 1024, 640
N_PHONEME, N_PHONE = 96, 230
N_CORES = 8
ROWS = B * T
RC = ROWS // N_CORES          # rows per core
NH = H // 128                 # contraction chunks
NB = RC // 128                # 128-row blocks per core
# row tiles: three 256-row tiles + two 128-row tiles (short tail)
TILE_RT = (2, 2, 2, 1, 1)
NTILES = len(TILE_RT)
NWARM = 28                    # PE-ramp warmup matmuls (128 cols each)
BF16 = ml_dtypes.bfloat16


def _structure(mapping: np.ndarray):
    """Segment-contiguous gather order, grouped by segment length (desc).

    Returns (col_ids, groups, perm):
      col_ids: phone index feeding each device matmul column (len NNZ)
      groups:  list of (L, nL, col_off, out_off) — nL segments of length L
               occupy matmul cols [col_off, col_off+nL*L) and device output
               cols [out_off, out_off+nL)
      perm:    perm[j] = original phoneme id of device output column j
    """
    segs = [np.nonzero(mapping[m] > 0)[0] for m in range(N_PHONEME)]
    assert min(len(s) for s in segs) >= 1
    # pad segment lengths up to even targets (repeating a member doesn't
    # change the max): fewer distinct lengths -> fewer DVE reduce ops.
    # Only worthwhile while the matmul width stays within one PSUM bank.
    padded = []
    for s in segs:
        t = ((len(s) + 1) // 2) * 2
        padded.append(np.concatenate([s, np.full(t - len(s), s[0], s.dtype)]))
    if sum(len(s) for s in padded) <= 512:
        segs = padded
    lengths = np.array([len(s) for s in segs])
    order = np.argsort(-lengths, kind="stable")
    col_ids, groups, perm = [], [], []
    i = 0
    while i < N_PHONEME:
        L = int(lengths[order[i]])
        j = i
        while j < N_PHONEME and lengths[order[j]] == L:
            j += 1
        groups.append((L, j - i, len(col_ids), i))
        for k in range(i, j):
            m = int(order[k])
            col_ids.extend(segs[m].tolist())
            perm.append(m)
        i = j
    return np.array(col_ids, dtype=np.int64), groups, np.array(perm, dtype=np.int64)


def _patch_act_tables():
    """Make Exp and Ln resolve to the same activation-table set.

    bacc's insert_act_table_loads models a single table slot, so a kernel
    alternating Exp/Ln reloads a 1.3us table on every transition.  act_info
    has a joint set ('natural_log_exp_and_others') containing both; keep the
    set list's order/indices intact but strip Exp/Ln from the other sets so
    the pass picks the joint set for both and emits a single load.
    """
    if getattr(bacc, "_act_tables_patched", False):
        return
    from concourse import hw_specs
    orig = hw_specs.get_activation_tables
    act = mybir.ActivationFunctionType

    def patched(module_arch):
        tabs = orig(module_arch)
        joint = [k for k, v in tabs.items() if act.Exp in v and act.Ln in v]
        if not joint:
            return tabs
        j = joint[0]
        return {
            k: (v if k == j else (v - {act.Exp, act.Ln}))
            for k, v in tabs.items()
        }

    bacc.get_activation_tables = patched
    bacc._act_tables_patched = True


def _build_program(nnz: int, groups):
    """Build + compile the per-core Bass program. Returns the Bacc object."""
    _patch_act_tables()
    nc = bacc.Bacc("TRN2", target_bir_lowering=False, debug=False)
    dt = mybir.dt
    act = mybir.ActivationFunctionType
    X = mybir.AxisListType.X

    # enc chunk-major per row tile; element (p, c, t) = enc[row0+t, c*128+p]
    enckA_d = nc.dram_tensor("encka", [3, 128, NH, 256], dt.bfloat16, kind="ExternalInput")
    enckB_d = nc.dram_tensor("enckb", [2, 128, NH, 128], dt.bfloat16, kind="ExternalInput")
    # W interleaved: [128, NH, nnz]; element (p, c, n) = W[c*128+p, n]
    wk_d = nc.dram_tensor("wk", [128, NH, nnz], dt.bfloat16, kind="ExternalInput")
    # out packed: [128, NB, 96]; element (p, b, m) = out[b*128+p, m]
    out_d = nc.dram_tensor("out", [128, NB, N_PHONEME], dt.float32, kind="ExternalOutput")

    with ExitStack() as ctx:
        tc = ctx.enter_context(tile.TileContext(nc))
        wpool = ctx.enter_context(tc.tile_pool(name="wpool", bufs=1))
        epool = ctx.enter_context(tc.tile_pool(name="epool", bufs=1))
        ppool = ctx.enter_context(tc.tile_pool(name="ppool", bufs=1, space="PSUM"))
        spool = ctx.enter_context(tc.tile_pool(name="spool", bufs=2))

        wt = wpool.tile([128, NH, nnz], dt.bfloat16)
        ets = [
            epool.tile([128, NH, 128 * rt], dt.bfloat16, tag=f"et{i}", name=f"et{i}")
            for i, rt in enumerate(TILE_RT)
        ]

        def dma_halves(dst, src, engines=(nc.sync, nc.scalar)):
            engines[0].dma_start(dst[0:64], src[0:64])
            engines[1].dma_start(dst[64:128], src[64:128])

        # Head DMAs: each 128-partition transfer split into partition halves
        # on the two HWDGE queues (descriptor-rate parallelism).  Weights
        # first (every matmul needs them), then enc tiles in use order.
        dma_halves(wt, wk_d)
        for i in range(NTILES):
            src = enckA_d[i] if i < 3 else enckB_d[i - 3]
            dma_halves(ets[i], src)

        # PE warmup: small dummy matmuls keep the tensor engine continuously
        # busy (ramping its p-state) until the real weights land.  They write
        # the first tile's PSUM bank; the real c==0 matmul (start=True)
        # overwrites it.
        wu = wpool.tile([128, 128], dt.bfloat16)
        nc.gpsimd.memset(wu[:], 0.0)
        pss = [
            ppool.tile([128, rt, 512], dt.float32, tag=f"ps{rt}", bufs=3 if rt == 2 else 2,
                       name=f"ps{i}")
            for i, rt in enumerate(TILE_RT)
        ]
        for _ in range(NWARM):
            nc.tensor.matmul(pss[0][:, 0, :128], wu[:], wu[:], start=True, stop=True)

        out_row = 0
        for i, rt in enumerate(TILE_RT):
            ps, et = pss[i], ets[i]
            for r in range(rt):
                for c in range(NH):
                    nc.tensor.matmul(
                        ps[:, r, :nnz],
                        et[:, c, r * 128:(r + 1) * 128],
                        wt[:, c, :],
                        start=(c == 0),
                        stop=(c == NH - 1),
                    )
            # segment max: one strided DVE reduce per length group
            pmax = spool.tile([128, rt, N_PHONEME], dt.float32, tag=f"pmax{rt}",
                              name=f"pmax{i}")
            for (L, nL, coff, ooff) in groups:
                src = ps[:, :, coff:coff + nL * L].rearrange(
                    "p r (s l) -> p r s l", l=L)
                nc.vector.reduce_max(pmax[:, :, ooff:ooff + nL], src, axis=X)
            # log-softmax: exp -> row sums -> -lse = Ln(1/sum) -> x + (-lse)
            ex = spool.tile([128, rt, N_PHONEME], dt.float32, tag=f"ex{rt}",
                            name=f"ex{i}")
            nc.scalar.activation(ex[:], pmax[:], act.Exp)
            se = spool.tile([128, rt], dt.float32, tag=f"se{rt}", name=f"se{i}")
            nc.vector.reduce_sum(se[:], ex[:], axis=X)
            rse = spool.tile([128, rt], dt.float32, tag=f"rse{rt}", name=f"rse{i}")
            nc.vector.reciprocal(rse[:], se[:])
            rls = spool.tile([128, rt], dt.float32, tag=f"rls{rt}", name=f"rls{i}")
            nc.scalar.activation(rls[:], rse[:], act.Ln)
            ott = spool.tile([128, rt, N_PHONEME], dt.float32, tag=f"ott{rt}",
                             name=f"ott{i}")
            for r in range(rt):
                nc.scalar.activation(ott[:, r, :], pmax[:, r, :], act.Identity,
                                     bias=rls[:, r:r + 1])
            dst = out_d[:, out_row:out_row + rt, :]
            if i == 0:
                # first tile's out on the (otherwise idle) GpSimd queue so the
                # two HWDGE queues keep their descriptor budget for enc
                nc.gpsimd.dma_start(dst, ott[:])
            else:
                dma_halves(dst, ott[:])
            out_row += rt

    nc.compile()
    return nc


_CACHE: dict = {}


def _get_compiled(mapping: np.ndarray):
    key = mapping.astype(np.float32).tobytes()
    if _CACHE.get("key") != key:
        col_ids, groups, perm = _structure(mapping)
        nc = _build_program(len(col_ids), groups)
        _CACHE.update(key=key, col_ids=col_ids, groups=groups, perm=perm, nc=nc)
    return _CACHE["nc"], _CACHE["col_ids"], _CACHE["perm"]


def _prep_in_maps(enc_output, feature2phone, col_ids):
    scale = np.float32(1.0) / np.sqrt(np.float32(H))
    wg = (feature2phone.astype(np.float32) * scale)[:, col_ids].astype(BF16)
    # [H, nnz] -> [128, NH, nnz]
    wk = np.ascontiguousarray(wg.reshape(NH, 128, -1).transpose(1, 0, 2))
    # enc [ROWS, H] -> per-core chunk-major row tiles
    e4 = enc_output.astype(BF16).reshape(N_CORES, NB, 128, NH, 128)
    # [core, block, t, c, p] -> [core, block, p, c, t]
    e4 = np.ascontiguousarray(e4.transpose(0, 1, 4, 3, 2))
    in_maps = []
    for cc in range(N_CORES):
        blk = e4[cc]  # [NB, 128, NH, 128]
        # three 256-row tiles: merge block pairs (0,1),(2,3),(4,5) on the row axis
        ea = np.stack([
            np.concatenate([blk[2 * j], blk[2 * j + 1]], axis=2) for j in range(3)
        ])  # [3, 128, NH, 256]
        eb = blk[6:8]  # [2, 128, NH, 128]
        in_maps.append({
            "encka": np.ascontiguousarray(ea),
            "enckb": np.ascontiguousarray(eb),
            "wk": wk,
        })
    return in_maps


def run_device(enc_output, feature2phone, mapping, trace=False, **kw):
    """Build/compile (cached), run on the 8 cores, return (output, BassKernelResults)."""
    enc_output = np.asarray(enc_output)
    feature2phone = np.asarray(feature2phone)
    mapping = np.asarray(mapping)
    nc, col_ids, perm = _get_compiled(mapping)
    in_maps = _prep_in_maps(enc_output, feature2phone, col_ids)
    res = run_bass_kernel_spmd(
        nc, in_maps, core_ids=list(range(N_CORES)), trace=trace, **kw
    )
    # device out [128, NB, 96] packed -> rows b*128+p
    dev = np.concatenate(
        [res.results[c]["out"].transpose(1, 0, 2).reshape(RC, N_PHONEME)
         for c in range(N_CORES)],
        axis=0,
    )
    out = np.empty_like(dev)
    out[:, perm] = dev
    return out.reshape(B, T, N_PHONEME).astype(np.float32), res


def kernel(enc_output, feature2phone, mapping):
    out, _ = run_device(enc_output, feature2phone, mapping)
    return out


# revision 7
# speedup vs baseline: 1.3638x; 1.1131x over previous
"""Trainium2 Bass kernel for CompositionalPhoneticsModel (segment_reduce).

Computation (reference):
    phone   = einsum('bth,hp->btp', enc_output, feature2phone) / sqrt(H)
    allo    = where(mapping>0, phone[:,:,None,:]*mapping, -inf)   # mapping is 0/1
    phoneme = max(allo, axis=-1)                                  # masked segment max
    out     = log_softmax(phoneme, axis=2)

Device strategy (8 NeuronCores, data-parallel over the B*T=8192 rows):
  * Host gathers feature2phone columns into segment-contiguous order
    (phones in 2 segments get duplicated columns), pads every segment to a
    length in {4,6,8,10} (512 matmul columns exactly = one PSUM bank, and
    only 4 strided DVE reduce_max ops per row tile), and folds in the
    1/sqrt(H) scale.  The device phoneme order is a permutation of 0..95;
    max/logsumexp are permutation-invariant so the host un-permutes at the
    end.
  * DMA on TRN2 is descriptor-limited per queue (~128-desc transfer = 1.3-2us
    regardless of bytes).  Three queues run in parallel: the two HWDGE
    queues (Sync/Scalar) carry the weights + big enc tiles as partition
    halves, and the GpSimd SWDGE queue — whose software descriptor
    generation coalesces multiple partitions per descriptor — carries the
    small enc tiles and all outputs.
  * enc is pre-cast to bf16 and pre-interleaved per row-tile as
    [128, NH, rows] (chunk-major) so per-partition lines are contiguous.
  * The PE p-state ramps with CONTINUOUS use (~3.7us to full speed; an idle
    gap resets it).  A stream of small warmup matmuls keeps the PE busy
    from kernel start until the real weights arrive.
  * Row tiles (1,2,2,2,1)*128 rows: a small first tile starts the DVE's
    segment-max pipeline ~1.5us earlier, a small last tile halves the
    exposed tail chain.
  * log-softmax without max-subtraction (phone logits are ~N(0,1); exp fits
    fp32 comfortably): Exp on ScalarE (bf16 out -> 2x DVE row sums), Ln and
    negate on ScalarE, final x - lse as ScalarE Identity activation with
    per-partition bias (Identity/Exp/Ln share one activation table).
"""

from contextlib import ExitStack

import numpy as np
import ml_dtypes

import concourse.bass as bass
import concourse.bacc as bacc
import concourse.tile as tile
from concourse import mybir
from concourse.bass_utils import run_bass_kernel_spmd

B, T, H = 8, 1024, 640
N_PHONEME, N_PHONE = 96, 230
N_CORES = 8
ROWS = B * T
RC = ROWS // N_CORES          # rows per core
NH = H // 128                 # contraction chunks
NB = RC // 128                # 128-row blocks per core
TILE_RT = (1, 2, 2, 2, 1)     # row blocks per tile (small head + tail tiles)
NTILES = len(TILE_RT)
NWARM = 24                    # PE-ramp warmup matmuls (128 cols each)
BF16 = ml_dtypes.bfloat16
PAD_LENGTHS = (4, 6, 8, 10)   # segment lengths after padding


def _structure(mapping: np.ndarray):
    """Segment-contiguous gather order, grouped by padded length (desc).

    Returns (col_ids, groups, perm):
      col_ids: phone index feeding each device matmul column (len NNZ)
      groups:  list of (L, nL, col_off, out_off) — nL segments of length L
               occupy matmul cols [col_off, col_off+nL*L) and device output
               cols [out_off, out_off+nL)
      perm:    perm[j] = original phoneme id of device output column j
    """
    segs = [np.nonzero(mapping[m] > 0)[0] for m in range(N_PHONEME)]
    assert min(len(s) for s in segs) >= 1
    # pad segment lengths up to the next target (repeating a member doesn't
    # change the max): fewer distinct lengths -> fewer DVE reduce ops.
    # Only worthwhile while the matmul width stays within one PSUM bank.
    for targets in (PAD_LENGTHS, (2, 4, 6, 8, 10), None):
        if targets is None:
            padded = segs
            break
        padded = []
        for s in segs:
            t = next(t for t in targets if t >= len(s))
            padded.append(np.concatenate([s, np.full(t - len(s), s[0], s.dtype)]))
        if sum(len(s) for s in padded) <= 512:
            break
    segs = padded
    lengths = np.array([len(s) for s in segs])
    order = np.argsort(-lengths, kind="stable")
    col_ids, groups, perm = [], [], []
    i = 0
    while i < N_PHONEME:
        L = int(lengths[order[i]])
        j = i
        while j < N_PHONEME and lengths[order[j]] == L:
            j += 1
        groups.append((L, j - i, len(col_ids), i))
        for k in range(i, j):
            m = int(order[k])
            col_ids.extend(segs[m].tolist())
            perm.append(m)
        i = j
    return np.array(col_ids, dtype=np.int64), groups, np.array(perm, dtype=np.int64)


def _patch_act_tables():
    """Make Exp and Ln resolve to the same activation-table set.

    bacc's insert_act_table_loads models a single table slot, so a kernel
    alternating Exp/Ln reloads a 1.3us table on every transition.  act_info
    has a joint set ('natural_log_exp_and_others') containing both; keep the
    set list's order/indices intact but strip Exp/Ln from the other sets so
    the pass picks the joint set for both and emits a single load.
    """
    if getattr(bacc, "_act_tables_patched", False):
        return
    from concourse import hw_specs
    orig = hw_specs.get_activation_tables
    act = mybir.ActivationFunctionType

    def patched(module_arch):
        tabs = orig(module_arch)
        joint = [k for k, v in tabs.items() if act.Exp in v and act.Ln in v]
        if not joint:
            return tabs
        j = joint[0]
        return {
            k: (v if k == j else (v - {act.Exp, act.Ln}))
            for k, v in tabs.items()
        }

    bacc.get_activation_tables = patched
    bacc._act_tables_patched = True


def _build_program(nnz: int, groups):
    """Build + compile the per-core Bass program. Returns the Bacc object."""
    _patch_act_tables()
    nc = bacc.Bacc("TRN2", target_bir_lowering=False, debug=False)
    dt = mybir.dt
    act = mybir.ActivationFunctionType
    X = mybir.AxisListType.X

    # enc chunk-major per row tile; element (p, c, t) = enc[row0+t, c*128+p]
    enckA_d = nc.dram_tensor("encka", [3, 128, NH, 256], dt.bfloat16, kind="ExternalInput")
    enckB_d = nc.dram_tensor("enckb", [2, 128, NH, 128], dt.bfloat16, kind="ExternalInput")
    # W interleaved: [128, NH, nnz]; element (p, c, n) = W[c*128+p, n]
    wk_d = nc.dram_tensor("wk", [128, NH, nnz], dt.bfloat16, kind="ExternalInput")
    # out packed: [128, NB, 96]; element (p, b, m) = out[b*128+p, m]
    out_d = nc.dram_tensor("out", [128, NB, N_PHONEME], dt.float32, kind="ExternalOutput")

    # tile i -> source slice: B-tensor holds the two 128-row tiles (0, last)
    esrc = [enckB_d[0], enckA_d[0], enckA_d[1], enckA_d[2], enckB_d[1]]

    with ExitStack() as ctx:
        tc = ctx.enter_context(tile.TileContext(nc))
        wpool = ctx.enter_context(tc.tile_pool(name="wpool", bufs=1))
        epool = ctx.enter_context(tc.tile_pool(name="epool", bufs=1))
        ppool = ctx.enter_context(tc.tile_pool(name="ppool", bufs=1, space="PSUM"))
        spool = ctx.enter_context(tc.tile_pool(name="spool", bufs=2))

        wt = wpool.tile([128, NH, nnz], dt.bfloat16)
        ets = [
            epool.tile([128, NH, 128 * rt], dt.bfloat16, tag=f"et{i}", name=f"et{i}")
            for i, rt in enumerate(TILE_RT)
        ]

        # Head DMAs across three queues:
        #   Sync q / Scalar q: weight halves, then the 256-row enc tiles as
        #     partition halves (descriptor-rate parallelism)
        #   GpSimd SWDGE q: the two 128-row enc tiles (coalesced descriptors),
        #     later all outputs
        nc.sync.dma_start(wt[0:64], wk_d[0:64])
        nc.scalar.dma_start(wt[64:128], wk_d[64:128])
        nc.gpsimd.dma_start(ets[0][:], esrc[0])
        for i in (1, 2, 3):
            nc.sync.dma_start(ets[i][0:64], esrc[i][0:64])
            nc.scalar.dma_start(ets[i][64:128], esrc[i][64:128])
        nc.gpsimd.dma_start(ets[4][:], esrc[4])

        # PE warmup: small dummy matmuls keep the tensor engine continuously
        # busy (ramping its p-state) until the real weights land.  They write
        # the first tile's PSUM bank; the real c==0 matmul (start=True)
        # overwrites it.
        wu = wpool.tile([128, 128], dt.bfloat16)
        nc.gpsimd.memset(wu[:], 0.0)
        pss = [
            ppool.tile([128, rt, 512], dt.float32, tag=f"ps{rt}",
                       bufs=3 if rt == 2 else 2, name=f"ps{i}")
            for i, rt in enumerate(TILE_RT)
        ]
        for _ in range(NWARM):
            nc.tensor.matmul(pss[0][:, 0, :128], wu[:], wu[:], start=True, stop=True)

        out_row = 0
        for i, rt in enumerate(TILE_RT):
            ps, et = pss[i], ets[i]
            for r in range(rt):
                for c in range(NH):
                    nc.tensor.matmul(
                        ps[:, r, :nnz],
                        et[:, c, r * 128:(r + 1) * 128],
                        wt[:, c, :],
                        start=(c == 0),
                        stop=(c == NH - 1),
                    )
            # segment max: one strided DVE reduce per length group
            pmax = spool.tile([128, rt, N_PHONEME], dt.float32, tag=f"pmax{rt}",
                              name=f"pmax{i}")
            for (L, nL, coff, ooff) in groups:
                src = ps[:, :, coff:coff + nL * L].rearrange(
                    "p r (s l) -> p r s l", l=L)
                nc.vector.reduce_max(pmax[:, :, ooff:ooff + nL], src, axis=X)
            # log-softmax: exp (bf16) -> row sums (DVE 2x) -> lse -> -lse ->
            # x + (-lse) via Identity with per-partition bias
            ex = spool.tile([128, rt, N_PHONEME], dt.bfloat16, tag=f"ex{rt}",
                            name=f"ex{i}")
            nc.scalar.activation(ex[:], pmax[:], act.Exp)
            se = spool.tile([128, rt], dt.float32, tag=f"se{rt}", name=f"se{i}")
            nc.vector.reduce_sum(se[:], ex[:], axis=X)
            lse = spool.tile([128, rt], dt.float32, tag=f"lse{rt}", name=f"lse{i}")
            nc.scalar.activation(lse[:], se[:], act.Ln)
            nls = spool.tile([128, rt], dt.float32, tag=f"nls{rt}", name=f"nls{i}")
            nc.scalar.activation(nls[:], lse[:], act.Identity, scale=-1.0)
            ott = spool.tile([128, rt, N_PHONEME], dt.float32, tag=f"ott{rt}",
                             name=f"ott{i}")
            for r in range(rt):
                nc.scalar.activation(ott[:, r, :], pmax[:, r, :], act.Identity,
                                     bias=nls[:, r:r + 1])
            nc.gpsimd.dma_start(out_d[:, out_row:out_row + rt, :], ott[:])
            out_row += rt

    nc.compile()
    return nc


_CACHE: dict = {}


def _get_compiled(mapping: np.ndarray):
    key = mapping.astype(np.float32).tobytes()
    if _CACHE.get("key") != key:
        col_ids, groups, perm = _structure(mapping)
        nc = _build_program(len(col_ids), groups)
        _CACHE.update(key=key, col_ids=col_ids, groups=groups, perm=perm, nc=nc)
    return _CACHE["nc"], _CACHE["col_ids"], _CACHE["perm"]


def _prep_in_maps(enc_output, feature2phone, col_ids):
    scale = np.float32(1.0) / np.sqrt(np.float32(H))
    wg = (feature2phone.astype(np.float32) * scale)[:, col_ids].astype(BF16)
    # [H, nnz] -> [128, NH, nnz]
    wk = np.ascontiguousarray(wg.reshape(NH, 128, -1).transpose(1, 0, 2))
    # enc [ROWS, H] -> per-core chunk-major row blocks [NB, 128, NH, 128]
    e4 = enc_output.astype(BF16).reshape(N_CORES, NB, 128, NH, 128)
    e4 = np.ascontiguousarray(e4.transpose(0, 1, 4, 3, 2))
    in_maps = []
    for cc in range(N_CORES):
        blk = e4[cc]  # [NB, 128, NH, 128]; tiles: (0), (1,2), (3,4), (5,6), (7)
        ea = np.stack([
            np.concatenate([blk[2 * j + 1], blk[2 * j + 2]], axis=2)
            for j in range(3)
        ])  # [3, 128, NH, 256]
        eb = np.stack([blk[0], blk[7]])  # [2, 128, NH, 128]
        in_maps.append({
            "encka": np.ascontiguousarray(ea),
            "enckb": np.ascontiguousarray(eb),
            "wk": wk,
        })
    return in_maps


def run_device(enc_output, feature2phone, mapping, trace=False, **kw):
    """Build/compile (cached), run on the 8 cores, return (output, BassKernelResults)."""
    enc_output = np.asarray(enc_output)
    feature2phone = np.asarray(feature2phone)
    mapping = np.asarray(mapping)
    nc, col_ids, perm = _get_compiled(mapping)
    in_maps = _prep_in_maps(enc_output, feature2phone, col_ids)
    res = run_bass_kernel_spmd(
        nc, in_maps, core_ids=list(range(N_CORES)), trace=trace, **kw
    )
    # device out [128, NB, 96] packed -> rows b*128+p
    dev = np.concatenate(
        [res.results[c]["out"].transpose(1, 0, 2).reshape(RC, N_PHONEME)
         for c in range(N_CORES)],
        axis=0,
    )
    out = np.empty_like(dev)
    out[:, perm] = dev
    return out.reshape(B, T, N_PHONEME).astype(np.float32), res


def kernel(enc_output, feature2phone, mapping):
    out, _ = run_device(enc_output, feature2phone, mapping)
    return out
